# revision 11
# baseline (speedup 1.0000x reference)
"""Trainium2 Bass kernel for nn_CausalTransformer (B=2, T=2048, L=6, E=256, H=4, DH=64, M=512).

Sharding: 8 cores = 2 (batch) x 4 (attention head). Per core:
  - full residual stream x^T [E, T] for its batch element (dense parts replicated
    within the 4-core group),
  - attention computed only for its own head,
  - one AllGather per half-sequence per layer combines the 4 heads' attention
    outputs (group of 4), after which out-proj / MLP run replicated.

Layout trick: residual kept transposed (x^T: E on partitions, tokens on free axis)
so no PE transposes are needed anywhere. Attention computes S^T = K^T.T @ Q^T
directly; softmax denominator comes free from a ones-column appended to V; the
relative-position bias + causal mask are added from a host-precomputed sliding
diagonal table D2[128, 2560] (bias tile == SBUF slice of D2).

All matmuls run in float32r (1-pass FP22) via bitcast.
"""

import math

import numpy as np

L, E, H, DH, M, NB, NA, OBS = 6, 256, 4, 64, 512, 32, 4, 637
MAX_DIST = 128
B, T, P = 2, 2048, 128
ET = E // P            # 2 e-tiles (E on partitions)
TT = T // P            # 16 token tiles
CH = 512               # matmul free-dim chunk
NCH = T // CH          # 4 chunks
KSE = E // P           # contraction subtiles over E
OBSP = 640             # obs dim padded to 5*128
KSO = OBSP // P        # 5
MT = M // P            # 4 m-tiles
EPS = 1.1920929e-07
NEG = -60000.0
XW = 2560              # D2 table width
NOUT = NA + OBS + 1    # 641 padded to 642 (even) for fp32r matmul
NCORES = 8

_CACHE = {}


def _build_program():
    import concourse.bass as bass
    import concourse.mybir as mybir
    import concourse.tile as tile
    from concourse import bacc

    f32 = mybir.dt.float32
    f32r = mybir.dt.float32r
    u32 = mybir.dt.uint32
    AF = mybir.ActivationFunctionType
    ALU = mybir.AluOpType
    ds = bass.ds

    def r(ap):
        return ap.bitcast(f32r)

    nc = bacc.Bacc("TRN2", target_bir_lowering=False, debug=False,
                   num_devices=NCORES)

    def din(name, shape, dt=f32):
        return nc.dram_tensor(name, shape, dt, kind="ExternalInput").ap()

    obsT = din("obsT", [OBSP, T])
    wobs = din("wobs", [OBSP, E])
    obsb = din("obsb", [P, ET])
    wqk = din("wqk", [L, E, 2 * DH])     # [wq*scale | wk] per layer, own head
    wv = din("wv", [L, E, DH])
    wo = din("wo", [L, E, E])
    w1 = din("w1", [L, E, M])
    w2 = din("w2", [L, M, E])
    b1r = din("b1r", [L, P, MT])
    b2r = din("b2r", [L, P, ET])
    n1w = din("n1w", [L, P, ET])
    n2w = din("n2w", [L, P, ET])
    d2t = din("d2t", [L, P, XW])
    fnw = din("fnw", [P, ET])
    whead = din("whead", [E, NOUT])
    bheadb = din("bheadb", [P, NOUT])
    toff = din("toff", [1, 1], u32)
    outq = nc.dram_tensor("outq", [CH, NOUT], f32, kind="ExternalOutput").ap()

    HALF = T // 2
    RG = [[0, 1, 2, 3], [4, 5, 6, 7]]

    with tile.TileContext(nc) as tc:
        from contextlib import ExitStack
        ctx = ExitStack()
        with ctx:
            xp = ctx.enter_context(tc.tile_pool(name="xp", bufs=1))
            hp = ctx.enter_context(tc.tile_pool(name="hp", bufs=1))
            qkp = ctx.enter_context(tc.tile_pool(name="qkp", bufs=1))
            vp = ctx.enter_context(tc.tile_pool(name="vp", bufs=1))
            esp = ctx.enter_context(tc.tile_pool(name="esp", bufs=5))
            onp = ctx.enter_context(tc.tile_pool(name="onp", bufs=1))
            otp = ctx.enter_context(tc.tile_pool(name="otp", bufs=1))
            rup = ctx.enter_context(tc.tile_pool(name="rup", bufs=2))
            sqp = ctx.enter_context(tc.tile_pool(name="sqp", bufs=3))
            rbp = ctx.enter_context(tc.tile_pool(name="rbp", bufs=2))
            smp = ctx.enter_context(tc.tile_pool(name="smp", bufs=2))
            wp = ctx.enter_context(tc.tile_pool(name="wp", bufs=2))
            cst = ctx.enter_context(tc.tile_pool(name="cst", bufs=1))
            drp = ctx.enter_context(tc.tile_pool(name="drp", bufs=2, space="DRAM"))
            ps_s = ctx.enter_context(tc.tile_pool(name="ps_s", bufs=2, space="PSUM"))
            ps_av = ctx.enter_context(tc.tile_pool(name="ps_av", bufs=2, space="PSUM"))
            ps_mm = ctx.enter_context(tc.tile_pool(name="ps_mm", bufs=2, space="PSUM"))
            ps_var = ctx.enter_context(tc.tile_pool(name="ps_var", bufs=2, space="PSUM"))

            # ---- constants / global tiles ----
            ones_f = cst.tile([P, 1], f32, name="ones_f")
            nc.gpsimd.memset(ones_f[:], 1.0)
            ones_c = cst.tile([P, 1], f32r, name="ones_c")
            nc.vector.tensor_copy(ones_c[:], ones_f[:])
            eps_c = cst.tile([P, 1], f32, name="eps_c")
            nc.gpsimd.memset(eps_c[:], EPS)
            whead_sb = cst.tile([P, KSE, NOUT], f32r, name="whead_sb")
            nc.sync.dma_start(whead_sb, whead.rearrange("(ks p) n -> p ks n", p=P).bitcast(f32r))
            bhead_sb = cst.tile([P, NOUT], f32, name="bhead_sb")
            nc.sync.dma_start(bhead_sb, bheadb)
            obsb_sb = cst.tile([P, ET], f32, name="obsb_sb")
            nc.sync.dma_start(obsb_sb, obsb)
            wobs_sb = cst.tile([P, KSO, E], f32r, name="wobs_sb")
            nc.sync.dma_start(wobs_sb, wobs.rearrange("(ks p) n -> p ks n", p=P).bitcast(f32r))

            # token-offset register for the final head phase
            tregs = nc.alloc_registers("toffr")
            nc.regs_load(tregs, toff[0:1, 0:1])
            sv = nc.snap(tregs, donate=True, min_val=0, max_val=T - CH)

            # persistent residual stream x^T
            x_sb = xp.tile([P, ET, T], f32, name="x_sb")

            def rmsnorm(src, dst, w_sb, width, tagp):
                """dst[:, et, :width] = src[:,et,:] * w[et] * rsqrt(mean_E(src^2)+eps).

                src/dst: [P, ET, width] APs. w_sb: [P, ET] tile.
                """
                nchunk = width // CH
                for c in range(nchunk):
                    sl = slice(c * CH, (c + 1) * CH)
                    vps = ps_var.tile([1, CH], f32, tag="var", name=f"vps{tagp}{c}")
                    for et in range(ET):
                        sq = sqp.tile([P, CH], f32r, tag="sq", name=f"sq{tagp}{c}{et}")
                        nc.vector.tensor_mul(sq, src[:, et, sl], src[:, et, sl])
                        nc.tensor.matmul(vps, r(ones_c[:]), r(sq[:]),
                                         start=(et == 0), stop=(et == ET - 1))
                    lnv = smp.tile([1, CH], f32, tag="lnv", name=f"lnv{tagp}{c}")
                    nc.scalar.activation(lnv, vps, AF.Ln, bias=eps_c[0:1, :],
                                         scale=1.0 / E)
                    rstd = smp.tile([1, CH], f32, tag="rstd", name=f"rstd{tagp}{c}")
                    nc.scalar.activation(rstd, lnv, AF.Exp, scale=-0.5)
                    rb = rbp.tile([P, CH], f32, tag="rb", name=f"rb{tagp}{c}")
                    nc.gpsimd.partition_broadcast(rb[:], rstd[:])
                    for et in range(ET):
                        nc.vector.scalar_tensor_tensor(
                            dst[:, et, sl], src[:, et, sl], w_sb[:, et:et + 1],
                            rb[:], ALU.mult, ALU.mult)

            # ---- obs projection: x^T = (obs @ Wobs + b)^T ----
            for c in range(NCH):
                sl = slice(c * CH, (c + 1) * CH)
                rhs_tiles = []
                for ks in range(KSO):
                    rt = esp.tile([P, CH], f32r, tag="es", name=f"obs{c}{ks}")
                    nc.sync.dma_start(rt, obsT[ks * P:(ks + 1) * P, sl].bitcast(f32r))
                    rhs_tiles.append(rt)
                for et in range(ET):
                    mps = ps_mm.tile([P, CH], f32, tag="mm", name=f"obsp{c}{et}")
                    for ks in range(KSO):
                        nc.tensor.matmul(
                            mps, r(wobs_sb[:, ks, et * P:(et + 1) * P]),
                            r(rhs_tiles[ks][:]),
                            start=(ks == 0), stop=(ks == KSO - 1))
                    nc.vector.tensor_scalar_add(x_sb[:, et, sl], mps,
                                                obsb_sb[:, et:et + 1])

            # ---- layers ----
            for l in range(L):
                wqk_sb = wp.tile([P, KSE, 2 * DH], f32r, tag="wqk", name=f"wqk{l}")
                nc.sync.dma_start(wqk_sb, wqk[l].rearrange("(ks p) n -> p ks n", p=P).bitcast(f32r))
                wv_sb = wp.tile([P, KSE, DH], f32r, tag="wv", name=f"wv{l}")
                nc.sync.dma_start(wv_sb, wv[l].rearrange("(ks p) n -> p ks n", p=P).bitcast(f32r))
                wo_sb = wp.tile([P, KSE, E], f32r, tag="wo", name=f"wo{l}")
                nc.sync.dma_start(wo_sb, wo[l].rearrange("(ks p) n -> p ks n", p=P).bitcast(f32r))
                w1_sb = wp.tile([P, KSE, M], f32r, tag="w1", name=f"w1{l}")
                nc.sync.dma_start(w1_sb, w1[l].rearrange("(ks p) n -> p ks n", p=P).bitcast(f32r))
                w2_sb = wp.tile([P, MT, E], f32r, tag="w2", name=f"w2{l}")
                nc.sync.dma_start(w2_sb, w2[l].rearrange("(ks p) n -> p ks n", p=P).bitcast(f32r))
                b1_sb = wp.tile([P, MT], f32, tag="b1", name=f"b1{l}")
                nc.sync.dma_start(b1_sb, b1r[l])
                b2_sb = wp.tile([P, ET], f32, tag="b2", name=f"b2{l}")
                nc.sync.dma_start(b2_sb, b2r[l])
                n1_sb = wp.tile([P, ET], f32, tag="n1", name=f"n1{l}")
                nc.sync.dma_start(n1_sb, n1w[l])
                n2_sb = wp.tile([P, ET], f32, tag="n2", name=f"n2{l}")
                nc.sync.dma_start(n2_sb, n2w[l])
                d2_sb = wp.tile([P, XW], f32, tag="d2", name=f"d2{l}", bufs=1)
                nc.sync.dma_start(d2_sb, d2t[l])

                # norm1
                h_sb = hp.tile([P, ET, T], f32r, tag="h", name=f"h{l}")
                rmsnorm(x_sb, h_sb, n1_sb, T, f"n1_{l}")

                # Q^T,K^T: [64,T] each
                qT = qkp.tile([64, T], f32r, tag="qT", name=f"qT{l}")
                kT = qkp.tile([64, T], f32r, tag="kT", name=f"kT{l}")
                for c in range(NCH):
                    sl = slice(c * CH, (c + 1) * CH)
                    for dst, c0 in ((qT, 0), (kT, DH)):
                        mps = ps_mm.tile([P, CH], f32, tag="mm",
                                         name=f"qk{l}{c}{c0}")
                        for ks in range(KSE):
                            nc.tensor.matmul(mps[0:64, :],
                                             r(wqk_sb[:, ks, c0:c0 + DH]),
                                             r(h_sb[:, ks, sl]),
                                             start=(ks == 0),
                                             stop=(ks == KSE - 1))
                        nc.vector.tensor_copy(dst[:, sl], mps[0:64, :])

                # V' tiles [P, tt, DH+1] (col DH = ones for the softmax denom)
                v_sb = vp.tile([P, TT, DH + 1], f32r, tag="v", name=f"v{l}")
                for tt in range(TT):
                    vps = ps_mm.tile([P, CH], f32, tag="mm", name=f"v{l}{tt}")
                    for ks in range(KSE):
                        nc.tensor.matmul(vps[:, :DH],
                                         r(h_sb[:, ks, tt * P:(tt + 1) * P]),
                                         r(wv_sb[:, ks, :]),
                                         start=(ks == 0), stop=(ks == KSE - 1))
                    nc.vector.tensor_copy(v_sb[:, tt, 0:DH], vps[:, :DH])
                    nc.vector.tensor_copy(v_sb[:, tt, DH:DH + 1], ones_c[:])

                # attention: S^T tiles -> +bias -> exp -> attnV accumulate
                outn = onp.tile([64, T], f32, tag="outn", name=f"outn{l}")
                jobs = [(qc, kt) for qc in range(NCH) for kt in range(4 * qc + 4)]
                DELAY = 2
                es_tiles = {}
                av_ps = {}

                def emit_s(qc, kt, l=l, d2_sb=d2_sb, qT=qT, kT=kT):
                    sps = ps_s.tile([P, CH], f32, tag="s", name=f"s{l}_{qc}_{kt}")
                    nc.tensor.matmul(sps, r(kT[:, kt * P:(kt + 1) * P]),
                                     r(qT[:, qc * CH:(qc + 1) * CH]),
                                     start=True, stop=True)
                    es = esp.tile([P, CH], f32r, tag="es", name=f"es{l}_{qc}_{kt}")
                    off = CH + qc * CH - kt * P
                    nc.vector.scalar_tensor_tensor(
                        es, sps, 1.0, d2_sb[:, off:off + CH], ALU.mult, ALU.add)
                    nc.scalar.activation(es, es, AF.Exp)
                    es_tiles[(qc, kt)] = es

                def emit_av(qc, kt, l=l, v_sb=v_sb, outn=outn):
                    if kt == 0:
                        av_ps[qc] = ps_av.tile([DH + 1, CH], f32, tag="av",
                                               name=f"av{l}_{qc}")
                    last = kt == 4 * qc + 3
                    nc.tensor.matmul(av_ps[qc], r(v_sb[:, kt, :]),
                                     r(es_tiles.pop((qc, kt))[:]),
                                     start=(kt == 0), stop=last,
                                     skip_group_check=True)
                    if last:
                        sl = slice(qc * CH, (qc + 1) * CH)
                        recip = smp.tile([1, CH], f32, tag="recip",
                                         name=f"recip{l}{qc}")
                        nc.vector.reciprocal(recip[:], av_ps[qc][DH:DH + 1, :])
                        rcb = rbp.tile([64, CH], f32, tag="rcb",
                                       name=f"rcb{l}{qc}")
                        nc.gpsimd.partition_broadcast(rcb[:], recip[:])
                        nc.vector.tensor_mul(outn[:, sl], av_ps[qc][0:DH, :],
                                             rcb[:])

                for i in range(len(jobs) + DELAY):
                    if i < len(jobs):
                        emit_s(*jobs[i])
                    if i >= DELAY:
                        emit_av(*jobs[i - DELAY])

                # AllGather heads per half-sequence, then out-proj + MLP
                outTall = {}
                for hf in range(2):
                    hsl = slice(hf * HALF, (hf + 1) * HALF)
                    agi = drp.tile([64, HALF], f32, tag="agi", name=f"agi{l}{hf}")
                    ago = drp.tile([E, HALF], f32, tag="ago", name=f"ago{l}{hf}")
                    nc.sync.dma_start(agi, outn[:, hsl])
                    nc.gpsimd.collective_compute(
                        "AllGather", ALU.bypass, replica_groups=RG,
                        ins=[agi.opt()], outs=[ago.opt()])
                    ot = otp.tile([P, KSE, HALF], f32r, tag="ot", name=f"ot{l}{hf}")
                    nc.sync.dma_start(ot, ago.rearrange("(ks p) t -> p ks t", p=P).bitcast(f32r))
                    outTall[hf] = ot

                for c in range(NCH):
                    sl = slice(c * CH, (c + 1) * CH)
                    ot = outTall[c // 2]
                    osl = slice((c % 2) * CH, (c % 2 + 1) * CH)
                    for et in range(ET):
                        mps = ps_mm.tile([P, CH], f32, tag="mm", name=f"wop{l}{c}{et}")
                        for ks in range(KSE):
                            nc.tensor.matmul(mps,
                                             r(wo_sb[:, ks, et * P:(et + 1) * P]),
                                             r(ot[:, ks, osl]),
                                             start=(ks == 0), stop=(ks == KSE - 1))
                        nc.vector.tensor_add(x_sb[:, et, sl], x_sb[:, et, sl], mps)

                # norm2 + MLP (chunked over tokens)
                h2_sb = hp.tile([P, ET, T], f32r, tag="h", name=f"h2{l}")
                rmsnorm(x_sb, h2_sb, n2_sb, T, f"n2_{l}")
                for c in range(NCH):
                    sl = slice(c * CH, (c + 1) * CH)
                    ru = rup.tile([P, MT, CH], f32r, tag="ru", name=f"ru{l}{c}")
                    for mt in range(MT):
                        mps = ps_mm.tile([P, CH], f32, tag="mm", name=f"up{l}{c}{mt}")
                        for ks in range(KSE):
                            nc.tensor.matmul(mps,
                                             r(w1_sb[:, ks, mt * P:(mt + 1) * P]),
                                             r(h2_sb[:, ks, sl]),
                                             start=(ks == 0), stop=(ks == KSE - 1))
                        nc.scalar.activation(ru[:, mt, :], mps, AF.Relu,
                                             bias=b1_sb[:, mt:mt + 1])
                    for et in range(ET):
                        mps = ps_mm.tile([P, CH], f32, tag="mm", name=f"dn{l}{c}{et}")
                        for mt in range(MT):
                            nc.tensor.matmul(mps,
                                             r(w2_sb[:, mt, et * P:(et + 1) * P]),
                                             r(ru[:, mt, :]),
                                             start=(mt == 0), stop=(mt == MT - 1))
                        nc.vector.scalar_tensor_tensor(
                            x_sb[:, et, sl], mps, b2_sb[:, et:et + 1],
                            x_sb[:, et, sl], ALU.add, ALU.add)

            # ---- final norm + heads on this core's token quarter ----
            fn_sb = cst.tile([P, ET], f32, name="fn_sb")
            nc.sync.dma_start(fn_sb, fnw)
            xq = xp.tile([P, ET, CH], f32, name="xq")
            for et in range(ET):
                nc.vector.tensor_copy(xq[:, et, :], x_sb[:, et, ds(sv, CH)])
            hq = xp.tile([P, ET, CH], f32r, name="hq")
            rmsnorm(xq, hq, fn_sb, CH, "fin")
            for tt in range(CH // P):
                oq = rup.tile([P, NOUT], f32, tag="oq", name=f"oq{tt}")
                for n0, nsz in ((0, CH), (CH, NOUT - CH)):
                    mps = ps_mm.tile([P, CH], f32, tag="mm", name=f"hd{tt}{n0}")
                    for ks in range(KSE):
                        nc.tensor.matmul(mps[:, :nsz],
                                         r(hq[:, ks, tt * P:(tt + 1) * P]),
                                         r(whead_sb[:, ks, n0:n0 + nsz]),
                                         start=(ks == 0), stop=(ks == KSE - 1))
                    nc.vector.tensor_add(oq[:, n0:n0 + nsz], mps[:, :nsz],
                                         bhead_sb[:, n0:n0 + nsz])
                nc.sync.dma_start(outq[tt * P:(tt + 1) * P, :], oq)

    nc.compile()
    return nc


def _bucket_table():
    """bucket(d) for d = q-k in [0, T), computed with jnp to match the oracle."""
    import jax.numpy as jnp
    d = jnp.arange(T, dtype=jnp.int32)
    nb = NB // 2
    buckets = (d > 0).astype(jnp.int32) * nb
    rel = jnp.abs(d)
    max_exact = nb // 2
    is_small = rel < max_exact
    large = max_exact + (
        jnp.log(jnp.maximum(rel, 1).astype(jnp.float32) / max_exact)
        / math.log(MAX_DIST / max_exact) * (nb - max_exact)
    ).astype(jnp.int32)
    large = jnp.minimum(large, nb - 1)
    return np.asarray(buckets + jnp.where(is_small, rel, large))


def _prep_in_maps(inputs):
    inp = {k: np.ascontiguousarray(np.asarray(v)) for k, v in inputs.items()}

    if "buckets" not in _CACHE:
        _CACHE["buckets"] = _bucket_table()
    buckets = _CACHE["buckets"]  # [T] int32, bucket(q-k) for q>=k

    f4 = np.float32
    scale = np.float32(DH ** -0.5)
    relb = inp["relb"]  # [L, NB, H]

    # per-head D2 sliding-bias tables: D2[i, x] = A[x - i + 128],
    # A[640 + d] = bias(d) (d in [0,T)), A elsewhere = NEG (causal mask)
    d2_by_head = []
    for h in range(H):
        d2l = np.empty((L, P, XW), f4)
        for l in range(L):
            A = np.full(XW + P, NEG, f4)
            A[640:640 + T] = relb[l][buckets, h]
            for i in range(P):
                d2l[l, i, :] = A[P - i:P - i + XW]
        d2_by_head.append(d2l)

    def col2(v):  # [E]->[P,ET] per-partition columns
        return np.ascontiguousarray(v.reshape(ET, P).T.astype(f4))

    wqk_by_head = []
    wv_by_head = []
    for h in range(H):
        hs = slice(h * DH, (h + 1) * DH)
        wqk_by_head.append(np.ascontiguousarray(np.concatenate(
            [inp["wq"][:, :, hs] * scale, inp["wk"][:, :, hs]], axis=2).astype(f4)))
        wv_by_head.append(np.ascontiguousarray(inp["wv"][:, :, hs].astype(f4)))

    obsT_by_b = []
    for b in range(B):
        ob = np.zeros((OBSP, T), f4)
        ob[:OBS] = inp["obs_seq"][b].T
        obsT_by_b.append(ob)

    wobs_p = np.zeros((OBSP, E), f4)
    wobs_p[:OBS] = inp["obs_proj_w"]
    b1r = np.ascontiguousarray(
        inp["b1"].reshape(L, MT, P).transpose(0, 2, 1).astype(f4))
    b2r = np.ascontiguousarray(
        inp["b2"].reshape(L, ET, P).transpose(0, 2, 1).astype(f4))
    n1w = np.ascontiguousarray(
        inp["norm1_w"].reshape(L, ET, P).transpose(0, 2, 1).astype(f4))
    n2w = np.ascontiguousarray(
        inp["norm2_w"].reshape(L, ET, P).transpose(0, 2, 1).astype(f4))
    whead = np.zeros((E, NOUT), f4)
    whead[:, :NA] = inp["act_w"]
    whead[:, NA:NA + OBS] = inp["obsh_w"]
    bhead = np.zeros(NOUT, f4)
    bhead[:NA] = inp["act_b"]
    bhead[NA:NA + OBS] = inp["obsh_b"]
    bheadb = np.ascontiguousarray(np.broadcast_to(bhead, (P, NOUT)))

    shared = dict(
        wobs=wobs_p, obsb=col2(inp["obs_proj_b"]),
        wo=inp["wo"].astype(f4), w1=inp["w1"].astype(f4),
        w2=inp["w2"].astype(f4), b1r=b1r, b2r=b2r, n1w=n1w, n2w=n2w,
        fnw=col2(inp["final_norm_w"]), whead=whead, bheadb=bheadb,
    )
    in_maps = []
    for c in range(NCORES):
        b, h = c // 4, c % 4
        m = dict(shared)
        m["obsT"] = obsT_by_b[b]
        m["wqk"] = wqk_by_head[h]
        m["wv"] = wv_by_head[h]
        m["d2t"] = d2_by_head[h]
        m["toff"] = np.array([[h * CH]], np.uint32)
        in_maps.append(m)
    return in_maps


def kernel(**inputs):
    from concourse.bass_utils import run_bass_kernel_spmd

    if "nc" not in _CACHE:
        _CACHE["nc"] = _build_program()
    nc = _CACHE["nc"]
    in_maps = _prep_in_maps(inputs)
    res = run_bass_kernel_spmd(nc, in_maps, core_ids=list(range(NCORES)))
    ys = []
    for b in range(B):
        yb = np.concatenate(
            [res.results[4 * b + qh]["outq"] for qh in range(4)], axis=0)
        ys.append(yb)
    y = np.stack(ys)  # [B, T, NOUT]
    return (np.ascontiguousarray(y[:, :, :NA]),
            np.ascontiguousarray(y[:, :, NA:NA + OBS]))


# revision 22
# speedup vs baseline: 1.1606x; 1.1606x over previous
"""Trainium2 Bass kernel for nn_CausalTransformer (B=2, T=2048, L=6, E=256, H=4, DH=64, M=512).

Sharding: 8 cores = 2 (batch) x 4 (attention head). Per core:
  - full residual stream x^T [E, T] for its batch element (dense parts replicated
    within the 4-core group),
  - attention computed only for its own head,
  - one AllGather per half-sequence per layer combines the 4 heads' attention
    outputs (group of 4), after which out-proj / MLP run replicated.

Layout tricks:
  - residual kept transposed (x^T: E on partitions, tokens free) -> no PE
    transposes anywhere; attention computes S^T = K^T.T @ Q^T directly.
  - softmax denominator comes free from a ones-column appended to V.
  - rel-pos bias + causal mask are a host-precomputed sliding diagonal table
    D2[128, 2560]; each bias tile is just an SBUF slice of it.
  - rmsnorm weights are folded into the consumer weight matrices host-side;
    the per-token 1/rms is applied at matmul copyback (Q/K/V) or on the
    matmul input (MLP), so no normalized copy of x is ever materialized.
  - AllGather latency is hidden: heads gathered per half-sequence, second
    half's attention runs while the first gather flies, dense work fills the
    second gather's window; next layer's norm1 stats run in the MLP tail.

All matmuls run in float32r (1-pass FP22).
"""

import math

import numpy as np

L, E, H, DH, M, NB, NA, OBS = 6, 256, 4, 64, 512, 32, 4, 637
MAX_DIST = 128
B, T, P = 2, 2048, 128
ET = E // P            # 2 e-tiles (E on partitions)
TT = T // P            # 16 token tiles
CH = 512               # matmul free-dim chunk
NCH = T // CH          # 4 chunks
KSE = E // P           # contraction subtiles over E
OBSP = 640             # obs dim padded to 5*128
KSO = OBSP // P        # 5
MT = M // P            # 4 m-tiles
EPS = 1.1920929e-07
NEG = -60000.0
XW = 2560              # D2 table width
NOUT = NA + OBS + 1    # 641 padded to 642 (even) for fp32r matmul
NCORES = 8

_CACHE = {}


def _build_program():
    import concourse.bass as bass
    import concourse.mybir as mybir
    import concourse.tile as tile
    from concourse import bacc

    f32 = mybir.dt.float32
    f32r = mybir.dt.float32r
    u32 = mybir.dt.uint32
    AF = mybir.ActivationFunctionType
    ALU = mybir.AluOpType
    ds = bass.ds

    def r(ap):
        return ap.bitcast(f32r)

    nc = bacc.Bacc("TRN2", target_bir_lowering=False, debug=False,
                   num_devices=NCORES)

    def din(name, shape, dt=f32):
        return nc.dram_tensor(name, shape, dt, kind="ExternalInput").ap()

    obsT = din("obsT", [OBSP, T])
    wobs = din("wobs", [OBSP, E])
    obsb = din("obsb", [P, ET])
    wqk = din("wqk", [L, E, 2 * DH])   # [n1w*(wq*scale) | n1w*wk], own head
    wv = din("wv", [L, E, DH])         # n1w-folded, own head
    wo = din("wo", [L, E, E])
    w1 = din("w1", [L, E, M])          # n2w-folded
    w2 = din("w2", [L, M, E])
    b1r = din("b1r", [L, P, MT])
    b2r = din("b2r", [L, P, ET])
    d2t = din("d2t", [L, P, XW])
    whead = din("whead", [E, NOUT])    # fnw-folded
    bheadb = din("bheadb", [P, NOUT])
    toff = din("toff", [1, 1], u32)
    outq = nc.dram_tensor("outq", [CH, NOUT], f32, kind="ExternalOutput").ap()
    import os as _os
    _kdbg = int(_os.environ.get("KDBG", "-1"))
    xdbg = (nc.dram_tensor("xdbg", [P, ET, T], f32, kind="ExternalOutput").ap()
            if _kdbg >= 0 else None)

    HALF = T // 2
    RG = [[0, 1, 2, 3], [4, 5, 6, 7]]

    with tile.TileContext(nc) as tc:
        from contextlib import ExitStack
        ctx = ExitStack()
        with ctx:
            xp = ctx.enter_context(tc.tile_pool(name="xp", bufs=1))
            hp = ctx.enter_context(tc.tile_pool(name="hp", bufs=3))
            qkp = ctx.enter_context(tc.tile_pool(name="qkp", bufs=1))
            vp = ctx.enter_context(tc.tile_pool(name="vp", bufs=1))
            esp = ctx.enter_context(tc.tile_pool(name="esp", bufs=6))
            onp = ctx.enter_context(tc.tile_pool(name="onp", bufs=1))
            otp = ctx.enter_context(tc.tile_pool(name="otp", bufs=2))
            rup = ctx.enter_context(tc.tile_pool(name="rup", bufs=2))
            sqp = ctx.enter_context(tc.tile_pool(name="sqp", bufs=3))
            rbp = ctx.enter_context(tc.tile_pool(name="rbp", bufs=2))
            smp = ctx.enter_context(tc.tile_pool(name="smp", bufs=2))
            wp = ctx.enter_context(tc.tile_pool(name="wp", bufs=2))
            cst = ctx.enter_context(tc.tile_pool(name="cst", bufs=1))
            drp = ctx.enter_context(tc.tile_pool(name="drp", bufs=2, space="DRAM"))
            ps_s = ctx.enter_context(tc.tile_pool(name="ps_s", bufs=3, space="PSUM"))
            ps_av = ctx.enter_context(tc.tile_pool(name="ps_av", bufs=2, space="PSUM"))
            ps_mm = ctx.enter_context(tc.tile_pool(name="ps_mm", bufs=2, space="PSUM"))
            ps_var = ctx.enter_context(tc.tile_pool(name="ps_var", bufs=1, space="PSUM"))

            # ---- constants / global tiles ----
            ones_f = cst.tile([P, 1], f32, name="ones_f")
            nc.gpsimd.memset(ones_f[:], 1.0)
            ones_c = cst.tile([P, 1], f32r, name="ones_c")
            nc.vector.tensor_copy(ones_c[:], ones_f[:])
            eps_c = cst.tile([P, 1], f32, name="eps_c")
            nc.gpsimd.memset(eps_c[:], EPS)
            whead_sb = cst.tile([P, KSE, NOUT], f32r, name="whead_sb")
            nc.sync.dma_start(whead_sb,
                              whead.rearrange("(ks p) n -> p ks n", p=P).bitcast(f32r))
            bhead_sb = cst.tile([P, NOUT], f32, name="bhead_sb")
            nc.sync.dma_start(bhead_sb, bheadb)
            obsb_sb = cst.tile([P, ET], f32, name="obsb_sb")
            nc.sync.dma_start(obsb_sb, obsb)
            wobs_sb = cst.tile([P, KSO, E], f32r, name="wobs_sb")
            nc.sync.dma_start(wobs_sb,
                              wobs.rearrange("(ks p) n -> p ks n", p=P).bitcast(f32r))

            # token-offset register for the final head phase
            tregs = nc.alloc_registers("toffr")
            nc.regs_load(tregs, toff[0:1, 0:1])
            sv = nc.snap(tregs, donate=True, min_val=0, max_val=T - CH)

            # persistent residual stream x^T
            x_sb = xp.tile([P, ET, T], f32r, name="x_sb")

            def norm_stats(src, width, tagp, want_rstdT=False):
                """rstd = rsqrt(mean_E(src^2)+eps) per token.

                Returns (rb, rstdT): rb [P, width] broadcast of rstd;
                rstdT [P, width//P] token-on-partition layout (if requested).
                src: [P, ET, width] AP (raw residual).
                Groups all Ln's then all Exp's (2 ACT table loads per call).
                """
                nchunk = width // CH
                lnrows = []
                for c in range(nchunk):
                    sl = slice(c * CH, (c + 1) * CH)
                    vps = ps_var.tile([1, CH], f32, tag="var", name=f"vps{tagp}{c}")
                    for et in range(ET):
                        sq = sqp.tile([P, CH], f32r, tag="sq",
                                      name=f"sq{tagp}{c}{et}")
                        nc.vector.tensor_mul(sq, src[:, et, sl], src[:, et, sl])
                        nc.tensor.matmul(vps, r(ones_c[:]), r(sq[:]),
                                         start=(et == 0), stop=(et == ET - 1))
                    lnrow = smp.tile([1, CH], f32, tag="lnv", name=f"ln{tagp}{c}")
                    nc.scalar.activation(lnrow, vps, AF.Ln, bias=eps_c[0:1, :],
                                         scale=1.0 / E)
                    lnrows.append(lnrow)
                rb = rbp.tile([P, width], f32, tag=f"rb{width}", name=f"rb{tagp}",
                              bufs=(1 if width == T else 2))
                rstds = []
                for c in range(nchunk):
                    rstd = smp.tile([1, CH], f32, tag="rstd", name=f"rs{tagp}{c}")
                    nc.scalar.activation(rstd, lnrows[c], AF.Exp, scale=-0.5)
                    rstds.append(rstd)
                for c in range(nchunk):
                    nc.gpsimd.partition_broadcast(
                        rb[:, c * CH:(c + 1) * CH], rstds[c][:])
                rstdT = None
                if want_rstdT:
                    rstdT = smp.tile([P, width // P], f32, tag="rstdT",
                                     name=f"rT{tagp}")
                    drt = drp.tile([1, width], f32, tag="rsd", name=f"dr{tagp}")
                    for c in range(nchunk):
                        nc.sync.dma_start(drt[:, c * CH:(c + 1) * CH],
                                          rstds[c][:])
                    nc.sync.dma_start(
                        rstdT, drt.rearrange("a (o p) -> p (a o)", p=P))
                return rb, rstdT

            # ---- obs projection: x^T = (obs @ Wobs + b)^T ----
            for c in range(NCH):
                sl = slice(c * CH, (c + 1) * CH)
                rhs_tiles = []
                for ks in range(KSO):
                    rt = esp.tile([P, CH], f32r, tag="es", name=f"obs{c}{ks}")
                    nc.sync.dma_start(rt, obsT[ks * P:(ks + 1) * P, sl].bitcast(f32r))
                    rhs_tiles.append(rt)
                for et in range(ET):
                    mps = ps_mm.tile([P, CH], f32, tag="mm", name=f"obsp{c}{et}")
                    for ks in range(KSO):
                        nc.tensor.matmul(
                            mps, r(wobs_sb[:, ks, et * P:(et + 1) * P]),
                            r(rhs_tiles[ks][:]),
                            start=(ks == 0), stop=(ks == KSO - 1))
                    nc.vector.tensor_scalar_add(x_sb[:, et, sl], mps,
                                                obsb_sb[:, et:et + 1])
            if _kdbg == 0:
                nc.sync.dma_start(xdbg, x_sb.bitcast(f32))
            n1 = norm_stats(x_sb, T, "n10", want_rstdT=True)

            # ---- layers ----
            for l in range(L):
                rb1, rstdT = n1
                wqk_sb = wp.tile([P, KSE, 2 * DH], f32r, tag="wqk", name=f"wqk{l}")
                nc.sync.dma_start(
                    wqk_sb, wqk[l].rearrange("(ks p) n -> p ks n", p=P).bitcast(f32r))
                wv_sb = wp.tile([P, KSE, DH], f32r, tag="wv", name=f"wv{l}")
                nc.sync.dma_start(
                    wv_sb, wv[l].rearrange("(ks p) n -> p ks n", p=P).bitcast(f32r))
                wo_sb = wp.tile([P, KSE, E], f32r, tag="wo", name=f"wo{l}")
                nc.sync.dma_start(
                    wo_sb, wo[l].rearrange("(ks p) n -> p ks n", p=P).bitcast(f32r))
                w1_sb = wp.tile([P, KSE, M], f32r, tag="w1", name=f"w1{l}")
                nc.sync.dma_start(
                    w1_sb, w1[l].rearrange("(ks p) n -> p ks n", p=P).bitcast(f32r))
                w2_sb = wp.tile([P, MT, E], f32r, tag="w2", name=f"w2{l}")
                nc.sync.dma_start(
                    w2_sb, w2[l].rearrange("(ks p) n -> p ks n", p=P).bitcast(f32r))
                b1_sb = wp.tile([P, MT], f32, tag="b1", name=f"b1{l}")
                nc.sync.dma_start(b1_sb, b1r[l])
                b2_sb = wp.tile([P, ET], f32, tag="b2", name=f"b2{l}")
                nc.sync.dma_start(b2_sb, b2r[l])
                d2_sb = wp.tile([P, XW], f32, tag="d2", name=f"d2{l}", bufs=1)
                nc.sync.dma_start(d2_sb, d2t[l])

                # Q^T,K^T [64,T] = rstd * (x @ w'): scale on copyback
                qT = qkp.tile([64, T], f32r, tag="qT", name=f"qT{l}")
                kT = qkp.tile([64, T], f32r, tag="kT", name=f"kT{l}")
                for c in range(NCH):
                    sl = slice(c * CH, (c + 1) * CH)
                    for dst, c0 in ((qT, 0), (kT, DH)):
                        mps = ps_mm.tile([P, CH], f32, tag="mm",
                                         name=f"qk{l}{c}{c0}")
                        for ks in range(KSE):
                            nc.tensor.matmul(mps[0:64, :],
                                             r(wqk_sb[:, ks, c0:c0 + DH]),
                                             r(x_sb[:, ks, sl]),
                                             start=(ks == 0),
                                             stop=(ks == KSE - 1))
                        nc.vector.tensor_mul(dst[:, sl], mps[0:64, :],
                                             rb1[0:64, sl])

                # V' tiles [P, tt, DH+1] (col DH = ones for the softmax denom)
                v_sb = vp.tile([P, TT, DH + 1], f32r, tag="v", name=f"v{l}")
                for tt in range(TT):
                    vps = ps_mm.tile([P, CH], f32, tag="mm", name=f"v{l}{tt}")
                    for ks in range(KSE):
                        nc.tensor.matmul(vps[:, :DH],
                                         r(x_sb[:, ks, tt * P:(tt + 1) * P]),
                                         r(wv_sb[:, ks, :]),
                                         start=(ks == 0), stop=(ks == KSE - 1))
                    nc.vector.tensor_scalar_mul(v_sb[:, tt, 0:DH], vps[:, :DH],
                                                rstdT[:, tt:tt + 1])
                    nc.vector.tensor_copy(v_sb[:, tt, DH:DH + 1], ones_c[:])

                if _kdbg == 13 and l == 0:
                    nc.sync.dma_start(xdbg[:, 0, 0:TT], rstdT)
                    nc.sync.dma_start(xdbg[:, 1, :], rb1)
                if _kdbg == 10 and l == 0:
                    nc.sync.dma_start(xdbg[0:64, 0, :], qT.bitcast(f32))
                    nc.sync.dma_start(xdbg[0:64, 1, :], kT.bitcast(f32))
                if _kdbg == 12 and l == 0:
                    nc.sync.dma_start(xdbg[:, 0, :TT * (DH + 1)],
                                      v_sb.bitcast(f32))
                # attention: S^T -> +bias -> exp -> attnV (ones col => denom)
                outn = onp.tile([64, T], f32, tag="outn", name=f"outn{l}")
                es_tiles = {}
                av_ps = {}

                def emit_s(qc, kt, l=l, d2_sb=d2_sb, qT=qT, kT=kT):
                    sps = ps_s.tile([P, CH], f32, tag="s", name=f"s{l}_{qc}_{kt}")
                    nc.tensor.matmul(sps, r(kT[:, kt * P:(kt + 1) * P]),
                                     r(qT[:, qc * CH:(qc + 1) * CH]),
                                     start=True, stop=True)
                    es = esp.tile([P, CH], f32r, tag="es", name=f"es{l}_{qc}_{kt}")
                    off = CH + qc * CH - kt * P
                    nc.vector.scalar_tensor_tensor(
                        es, sps, 1.0, d2_sb[:, off:off + CH], ALU.mult, ALU.add)
                    nc.scalar.activation(es, es, AF.Exp)
                    es_tiles[(qc, kt)] = es

                def emit_av(qc, kt, l=l, v_sb=v_sb, outn=outn):
                    if kt == 0:
                        av_ps[qc] = ps_av.tile([DH + 1, CH], f32, tag="av",
                                               name=f"av{l}_{qc}")
                    last = kt == 4 * qc + 3
                    nc.tensor.matmul(av_ps[qc], r(v_sb[:, kt, :]),
                                     r(es_tiles.pop((qc, kt))[:]),
                                     start=(kt == 0), stop=last,
                                     skip_group_check=True)
                    if last:
                        sl = slice(qc * CH, (qc + 1) * CH)
                        dnr = smp.tile([1, CH], f32, tag="dnr",
                                       name=f"dnr{l}{qc}")
                        nc.vector.tensor_copy(dnr[:], av_ps[qc][DH:DH + 1, :])
                        rsc = smp.tile([1, CH], f32, tag="rscr",
                                       name=f"rsc{l}{qc}")
                        nc.vector.reciprocal_approx_accurate(
                            dnr[:], dnr[:], rsc[:])
                        rcb = rbp.tile([64, CH], f32, tag="rcb",
                                       name=f"rcb{l}{qc}")
                        nc.gpsimd.partition_broadcast(rcb[:], dnr[:])
                        nc.vector.tensor_mul(outn[:, sl], av_ps[qc][0:DH, :],
                                             rcb[:])

                def attn_block(qcs, delay=2):
                    jobs = [(qc, kt) for qc in qcs for kt in range(4 * qc + 4)]
                    for i in range(len(jobs) + delay):
                        if i < len(jobs):
                            emit_s(*jobs[i])
                        if i >= delay:
                            emit_av(*jobs[i - delay])

                def emit_ag(hf, l=l, outn=outn):
                    hsl = slice(hf * HALF, (hf + 1) * HALF)
                    agi = drp.tile([64, HALF], f32, tag="agi", name=f"agi{l}{hf}")
                    ago = drp.tile([E, HALF], f32, tag="ago", name=f"ago{l}{hf}")
                    nc.sync.dma_start(agi, outn[:, hsl])
                    nc.gpsimd.collective_compute(
                        "AllGather", ALU.bypass, replica_groups=RG,
                        ins=[agi.opt()], outs=[ago.opt()])
                    ot = otp.tile([P, KSE, HALF], f32r, tag="ot", name=f"ot{l}{hf}")
                    nc.sync.dma_start(
                        ot, ago.rearrange("(ks p) t -> p ks t", p=P).bitcast(f32r))
                    return ot

                def emit_wo(c, ot, l=l, wo_sb=wo_sb):
                    sl = slice(c * CH, (c + 1) * CH)
                    osl = slice((c % 2) * CH, (c % 2 + 1) * CH)
                    for et in range(ET):
                        mps = ps_mm.tile([P, CH], f32, tag="mm",
                                         name=f"wop{l}{c}{et}")
                        for ks in range(KSE):
                            nc.tensor.matmul(mps,
                                             r(wo_sb[:, ks, et * P:(et + 1) * P]),
                                             r(ot[:, ks, osl]),
                                             start=(ks == 0), stop=(ks == KSE - 1))
                        nc.vector.tensor_add(x_sb[:, et, sl], x_sb[:, et, sl], mps)

                def emit_norm2(c, rb2s, l=l):
                    # rstd2 for chunk c (n2w folded into w1)
                    sl = slice(c * CH, (c + 1) * CH)
                    vps = ps_var.tile([1, CH], f32, tag="var", name=f"v2{l}{c}")
                    for et in range(ET):
                        sq = sqp.tile([P, CH], f32r, tag="sq", name=f"s2{l}{c}{et}")
                        nc.vector.tensor_mul(sq, x_sb[:, et, sl], x_sb[:, et, sl])
                        nc.tensor.matmul(vps, r(ones_c[:]), r(sq[:]),
                                         start=(et == 0), stop=(et == ET - 1))
                    lnrow = smp.tile([1, CH], f32, tag="lnv", name=f"l2{l}{c}")
                    nc.scalar.activation(lnrow, vps, AF.Ln, bias=eps_c[0:1, :],
                                         scale=1.0 / E)
                    rstd = smp.tile([1, CH], f32, tag="rstd", name=f"r2{l}{c}")
                    nc.scalar.activation(rstd, lnrow, AF.Exp, scale=-0.5)
                    rb2 = rbp.tile([P, CH], f32, tag="rb512", name=f"rb2{l}{c}")
                    nc.gpsimd.partition_broadcast(rb2[:], rstd[:])
                    rb2s[c] = rb2

                def emit_mlp(c, rb2s, l=l, w1_sb=w1_sb, w2_sb=w2_sb,
                             b1_sb=b1_sb, b2_sb=b2_sb):
                    sl = slice(c * CH, (c + 1) * CH)
                    h2 = hp.tile([P, ET, CH], f32r, tag="h2", name=f"h2{l}{c}")
                    for et in range(ET):
                        nc.vector.tensor_mul(h2[:, et, :], x_sb[:, et, sl],
                                             rb2s[c][:])
                    ru = rup.tile([P, MT, CH], f32r, tag="ru", name=f"ru{l}{c}")
                    for mt in range(MT):
                        mps = ps_mm.tile([P, CH], f32, tag="mm",
                                         name=f"up{l}{c}{mt}")
                        for ks in range(KSE):
                            nc.tensor.matmul(mps,
                                             r(w1_sb[:, ks, mt * P:(mt + 1) * P]),
                                             r(h2[:, ks, :]),
                                             start=(ks == 0), stop=(ks == KSE - 1))
                        nc.scalar.activation(ru[:, mt, :], mps, AF.Relu,
                                             bias=b1_sb[:, mt:mt + 1])
                    for et in range(ET):
                        mps = ps_mm.tile([P, CH], f32, tag="mm",
                                         name=f"dn{l}{c}{et}")
                        for mt in range(MT):
                            nc.tensor.matmul(mps,
                                             r(w2_sb[:, mt, et * P:(et + 1) * P]),
                                             r(ru[:, mt, :]),
                                             start=(mt == 0), stop=(mt == MT - 1))
                        nc.vector.scalar_tensor_tensor(
                            x_sb[:, et, sl], mps, b2_sb[:, et:et + 1],
                            x_sb[:, et, sl], ALU.add, ALU.add)

                # schedule: attention halves bracket the AGs; dense work fills
                # the AG-B window; next layer's norm1 runs in the MLP tail.
                attn_block([0, 1])
                ot_a = emit_ag(0)
                attn_block([2, 3])
                ot_b = emit_ag(1)
                if _kdbg == 11 and l == 0:
                    nc.sync.dma_start(xdbg[0:64, 0, :], outn)
                rb2s = {}
                emit_wo(0, ot_a)
                emit_wo(1, ot_a)
                emit_norm2(0, rb2s)
                emit_norm2(1, rb2s)
                emit_mlp(0, rb2s)
                emit_mlp(1, rb2s)
                emit_wo(2, ot_b)
                emit_wo(3, ot_b)
                emit_norm2(2, rb2s)
                emit_norm2(3, rb2s)
                emit_mlp(2, rb2s)
                emit_mlp(3, rb2s)
                if _kdbg == l + 1:
                    nc.sync.dma_start(xdbg, x_sb.bitcast(f32))
                if l + 1 < L:
                    n1 = norm_stats(x_sb, T, f"n1{l + 1}", want_rstdT=True)

            # ---- final norm + heads on this core's token quarter ----
            xq = xp.tile([P, ET, CH], f32, name="xq")
            for et in range(ET):
                nc.vector.tensor_copy(xq[:, et, :], x_sb[:, et, ds(sv, CH)])
            rbf, _ = norm_stats(xq, CH, "fin")
            hq = xp.tile([P, ET, CH], f32r, name="hq")
            for et in range(ET):
                nc.vector.tensor_mul(hq[:, et, :], xq[:, et, :], rbf[:])
            for tt in range(CH // P):
                oq = rup.tile([P, NOUT], f32, tag="oq", name=f"oq{tt}")
                for n0, nsz in ((0, CH), (CH, NOUT - CH)):
                    mps = ps_mm.tile([P, CH], f32, tag="mm", name=f"hd{tt}{n0}")
                    for ks in range(KSE):
                        nc.tensor.matmul(mps[:, :nsz],
                                         r(hq[:, ks, tt * P:(tt + 1) * P]),
                                         r(whead_sb[:, ks, n0:n0 + nsz]),
                                         start=(ks == 0), stop=(ks == KSE - 1))
                    nc.vector.tensor_add(oq[:, n0:n0 + nsz], mps[:, :nsz],
                                         bhead_sb[:, n0:n0 + nsz])
                nc.sync.dma_start(outq[tt * P:(tt + 1) * P, :], oq)

    nc.compile()
    return nc


def _bucket_table():
    """bucket(d) for d = q-k in [0, T), computed with jnp to match the oracle."""
    import jax.numpy as jnp
    d = jnp.arange(T, dtype=jnp.int32)
    nb = NB // 2
    buckets = (d > 0).astype(jnp.int32) * nb
    rel = jnp.abs(d)
    max_exact = nb // 2
    is_small = rel < max_exact
    large = max_exact + (
        jnp.log(jnp.maximum(rel, 1).astype(jnp.float32) / max_exact)
        / math.log(MAX_DIST / max_exact) * (nb - max_exact)
    ).astype(jnp.int32)
    large = jnp.minimum(large, nb - 1)
    return np.asarray(buckets + jnp.where(is_small, rel, large))


def _prep_in_maps(inputs):
    inp = {k: np.ascontiguousarray(np.asarray(v)) for k, v in inputs.items()}

    if "buckets" not in _CACHE:
        _CACHE["buckets"] = _bucket_table()
    buckets = _CACHE["buckets"]  # [T] int32, bucket(q-k) for q>=k

    f4 = np.float32
    scale = np.float32(DH ** -0.5)
    relb = inp["relb"]  # [L, NB, H]

    # per-head D2 sliding-bias tables: D2[i, x] = A[x - i + 128],
    # A[640 + d] = bias(d) (d in [0,T)), A elsewhere = NEG (causal mask)
    d2_by_head = []
    for h in range(H):
        d2l = np.empty((L, P, XW), f4)
        for l in range(L):
            A = np.full(XW + P, NEG, f4)
            A[640:640 + T] = relb[l][buckets, h]
            for i in range(P):
                d2l[l, i, :] = A[P - i:P - i + XW]
        d2_by_head.append(d2l)

    n1 = inp["norm1_w"].astype(f4)[:, :, None]     # [L, E, 1]
    n2 = inp["norm2_w"].astype(f4)[:, :, None]
    wqk_by_head = []
    wv_by_head = []
    for h in range(H):
        hs = slice(h * DH, (h + 1) * DH)
        wqk_by_head.append(np.ascontiguousarray(np.concatenate(
            [inp["wq"][:, :, hs] * scale * n1, inp["wk"][:, :, hs] * n1],
            axis=2).astype(f4)))
        wv_by_head.append(
            np.ascontiguousarray((inp["wv"][:, :, hs] * n1).astype(f4)))

    obsT_by_b = []
    for b in range(B):
        ob = np.zeros((OBSP, T), f4)
        ob[:OBS] = inp["obs_seq"][b].T
        obsT_by_b.append(ob)

    wobs_p = np.zeros((OBSP, E), f4)
    wobs_p[:OBS] = inp["obs_proj_w"]
    b1r = np.ascontiguousarray(
        inp["b1"].reshape(L, MT, P).transpose(0, 2, 1).astype(f4))
    b2r = np.ascontiguousarray(
        inp["b2"].reshape(L, ET, P).transpose(0, 2, 1).astype(f4))

    def col2(v):  # [E]->[P,ET] per-partition columns
        return np.ascontiguousarray(v.reshape(ET, P).T.astype(f4))

    fn = inp["final_norm_w"].astype(f4)[:, None]   # [E, 1]
    whead = np.zeros((E, NOUT), f4)
    whead[:, :NA] = inp["act_w"] * fn
    whead[:, NA:NA + OBS] = inp["obsh_w"] * fn
    bhead = np.zeros(NOUT, f4)
    bhead[:NA] = inp["act_b"]
    bhead[NA:NA + OBS] = inp["obsh_b"]
    bheadb = np.ascontiguousarray(np.broadcast_to(bhead, (P, NOUT)))

    shared = dict(
        wobs=wobs_p, obsb=col2(inp["obs_proj_b"]),
        wo=inp["wo"].astype(f4), w1=(inp["w1"] * n2).astype(f4),
        w2=inp["w2"].astype(f4), b1r=b1r, b2r=b2r,
        whead=whead, bheadb=bheadb,
    )
    in_maps = []
    for c in range(NCORES):
        b, h = c // 4, c % 4
        m = dict(shared)
        m["obsT"] = obsT_by_b[b]
        m["wqk"] = wqk_by_head[h]
        m["wv"] = wv_by_head[h]
        m["d2t"] = d2_by_head[h]
        m["toff"] = np.array([[h * CH]], np.uint32)
        in_maps.append(m)
    return in_maps


def kernel(**inputs):
    from concourse.bass_utils import run_bass_kernel_spmd

    if "nc" not in _CACHE:
        _CACHE["nc"] = _build_program()
    nc = _CACHE["nc"]
    in_maps = _prep_in_maps(inputs)
    res = run_bass_kernel_spmd(nc, in_maps, core_ids=list(range(NCORES)))
    ys = []
    for b in range(B):
        yb = np.concatenate(
            [res.results[4 * b + qh]["outq"] for qh in range(4)], axis=0)
        ys.append(yb)
    y = np.stack(ys)  # [B, T, NOUT]
    return (np.ascontiguousarray(y[:, :, :NA]),
            np.ascontiguousarray(y[:, :, NA:NA + OBS]))
